# revision 1
# baseline (speedup 1.0000x reference)
"""Trainium2 Bass kernel for nn_Attn_58669253263845 (sparse_attention).

Reference computation:
    hidden2 = concat(hidden[0], hidden[1])                 # [B, 2H]
    attn_input = concat(bcast(hidden2), encoder_outputs)   # [B, S, 3H]
    energy = attn_input @ W.T + b                          # [B, S, H]
    scores = energy @ v                                    # [B, S]
    out = softmax(scores, axis=S)

Everything before the softmax is linear, so
    scores[b,s] = attn_input[b,s,:] . (v @ W) + v.b
                = hidden2[b,:] . w_hid + enc[b,s,:] . w_enc + v.b
The hidden/bias terms are constant per batch row and cancel in the softmax
over S.  Hence:
    out = softmax_s(enc[b,s,:] . w_enc),  w_enc = v @ W[:, 2H:3H]

The weight fold (1024x1024 matvec) is done on host in fp64; the heavy part
(64*512 dot products of length 1024 + softmax) runs on 8 NeuronCores,
data-parallel over batch (8 batches per core).
"""

import sys
import types

import numpy as np
import concourse.bacc as bacc
import concourse.bass as bass
import concourse.mybir as mybir
import concourse.tile as tile
from concourse.bass_utils import run_bass_kernel_spmd

# run_bass_kernel_spmd(trace=True) (e.g. via BASS_TRACE=1 in the env)
# imports antenv.axon_hooks, which does not exist in this container. Register
# a stub returning "no hook" so tracing degrades gracefully instead of
# raising ModuleNotFoundError.
try:
    import antenv.axon_hooks  # noqa: F401
except ImportError:
    try:
        import antenv

        _stub = types.ModuleType("antenv.axon_hooks")
        _stub.get_axon_ntff_profile_hook = lambda: None  # type: ignore[attr-defined]
        sys.modules["antenv.axon_hooks"] = _stub
        antenv.axon_hooks = _stub
    except ImportError:
        pass

N_CORES = 8
B, S, H = 64, 512, 1024
P = 128            # SBUF partitions
BPC = B // N_CORES  # batches per core = 8
JT = S // P         # s-chunks per batch = 4

F32 = mybir.dt.float32

_compiled_nc = None
LAST_RESULTS = None  # BassKernelResults of the most recent run (for profiling)

# knobs (read at build time)
# One SBUF tile per chunk (~16MiB of the 24MiB SBUF): zero slot reuse means
# zero WAW waits -> no legalized EventSemaphore stalls on the DVE sequencer.
EBUF_BUFS = 38
TAIL_CHUNKS = 6  # how many trailing chunks get their dot split in half
LAST_SPLIT = 2  # pieces for the very last chunk (partial-tile width)
# Size of the final piece of the last chunk. 512 (symmetric halves) is
# optimal: sem-propagation (900ns) exceeds a half-chunk transfer (728ns),
# so shrinking the last piece below half only delays its predecessor.
LAST_PIECE = 512
SCORES_PAD = 8  # f32 per score-accumulator slot (32B dep-tracking granule)


def _build_nc(ebuf_bufs=None, dma_only=False, compute_only=False):
    """Per-core kernel: probs[BPC, S] = softmax_s(enc[BPC, S, H] @ w_enc).

    dma_only / compute_only build crippled variants for cost attribution.
    """
    # Bacc (not raw Bass): its compile() legalizes multi-wait instructions
    # into EventSemaphore waits (TRN2 allows only 1 sync wait per inst).
    nc = bacc.Bacc("TRN2", target_bir_lowering=False, debug=False)

    enc_d = nc.dram_tensor("enc_in", [BPC, S, H], F32, kind="ExternalInput")
    w_d = nc.dram_tensor("w_in", [1, H], F32, kind="ExternalInput")
    out_d = nc.dram_tensor("probs_out", [BPC, S], F32, kind="ExternalOutput")

    enc = enc_d.ap()

    with tile.TileContext(nc) as tc:
        with (
            tc.tile_pool(name="const", bufs=1) as constp,
            tc.tile_pool(name="ebuf", bufs=ebuf_bufs or EBUF_BUFS) as ebufp,
            tc.tile_pool(name="small", bufs=1) as smallp,
            tc.tile_pool(name="psum", bufs=1, space="PSUM") as psump,
        ):
            # The first enc chunk goes first in the DMA stream: every other
            # DMA's descriptor-gen then hides behind a running transfer.
            et0 = ebufp.tile([P, H], F32, name="et", tag="et")
            if compute_only:
                nc.sync.dma_start(et0[0:1, 0:1], enc[0, 0:1, 0:1])
            else:
                nc.sync.dma_start(et0[:], enc[0, 0:P, :])

            # w arrives as a single row (4KiB) and is broadcast to all 128
            # partitions on-device via a K=1 matmul with a ones row -- much
            # cheaper than DMAing a host-replicated 512KiB copy. It is then
            # copied once from PSUM to SBUF: PSUM dependency tracking is
            # bank-granular and serializes successive readers, so leaving w
            # in PSUM would chain every dot product to its predecessor (a
            # ~1.4us legalized wait on the DVE sequencer per chunk).
            w_row = constp.tile([1, H], F32, name="w_row")
            nc.sync.dma_start(w_row[:], w_d.ap())
            ones_t = constp.tile([1, P], F32, name="ones_t")
            nc.gpsimd.memset(ones_t[:], 1.0)
            w_ps = psump.tile([P, H], F32, name="w_ps")  # spans 2 PSUM banks
            half = H // 2
            nc.tensor.matmul(w_ps[:, 0:half], ones_t[:], w_row[:, 0:half])
            nc.tensor.matmul(w_ps[:, half:H], ones_t[:], w_row[:, half:H])
            w_t = constp.tile([P, H], F32, name="w_t")
            # also serves as the probe: DVE observes the PE broadcast here, so
            # the dots carry only their own DMA wait (TRN2 TPB instruction
            # structs only encode a single sync wait).
            nc.vector.tensor_copy(w_t[:], w_ps[:])

            # identity for the PE transposes, built on-device (gpsimd is idle
            # and this keeps 64KiB off the serial DMA stream):
            # ones everywhere, then keep only where p - f == 0.
            ones_id = constp.tile([P, P], F32, name="ones_id")
            nc.gpsimd.memset(ones_id[:], 1.0)
            id_t = constp.tile([P, P], F32, name="id_t")
            nc.gpsimd.affine_select(
                out=id_t[:],
                in_=ones_id[:],
                pattern=[[-1, P]],
                compare_op=mybir.AluOpType.is_equal,
                fill=0.0,
                channel_multiplier=1,
            )
            id_probe = psump.tile([1, 1], F32, name="id_probe")
            nc.tensor.matmul(id_probe[:], id_t[:, 0:1], id_t[:, 0:1])

            # scores[p, ci, 0] = enc[b, 128*j + p, :] . w_enc for chunk
            # ci = j*BPC + b. Each accumulator slot is padded to 32B (SCORES_PAD
            # f32): adjacent slots would otherwise share a dependency-tracking
            # granule, chaining every dot to its predecessor (WAW) and forcing
            # a ~1.4us legalized wait onto the DVE sequencer per chunk.
            scores = smallp.tile([P, JT * BPC, SCORES_PAD], F32, name="scores")

            # One 512KiB DMA + one fused dot per (b, j) chunk: finest natural
            # granularity, so compute trails the DMA stream by only one chunk.
            # The trailing TAIL_CHUNKS chunks are split in half along H so the
            # final (un-overlapped) DVE ops shrink: the second-to-last dot is
            # what actually gates the last one.
            # j-major chunk order: all 8 batches of column-group j arrive
            # consecutively, so transpose j + exp j overlap the remaining
            # dot-product stream for j < JT-1.
            chunks = [(j, b) for j in range(JT) for b in range(BPC)]
            nt = len(chunks)
            if TAIL_CHUNKS > 0:
                partial = smallp.tile(
                    [P, TAIL_CHUNKS, LAST_SPLIT], F32, name="partial"
                )
            for ci, (j, b) in enumerate(chunks):
                split = ci >= nt - TAIL_CHUNKS and not dma_only and not compute_only
                if not split:
                    slices = [(0, H)]
                elif ci == nt - 1:
                    # asymmetric: the second piece (the only fully exposed
                    # dot in the whole kernel) is as small as possible
                    slices = [(0, H - LAST_PIECE), (H - LAST_PIECE, LAST_PIECE)]
                else:
                    slices = [(0, H // 2), (H // 2, H // 2)]
                nsplit = len(slices)
                for h in range(nsplit):
                    h0, hs = slices[h]
                    if ci == 0 and nsplit == 1:
                        et = et0  # DMA already issued before the w block
                    else:
                        et = ebufp.tile([P, hs], F32, name="et", tag="et")
                        # enc[b, 128j:128(j+1), hslice] rows are contiguous
                        if compute_only:
                            nc.sync.dma_start(et[0:1, 0:1], enc[b, 0:1, 0:1])
                        else:
                            nc.sync.dma_start(
                                et[:],
                                enc[b, j * P : (j + 1) * P, h0 : h0 + hs],
                            )
                    if dma_only:
                        continue
                    # fused elementwise-mult + free-dim reduction on DVE:
                    # et *= w ; accum = sum(...)
                    # The product is written in place over the enc tile
                    # (dead after this op): no scratch tile / WAW waits.
                    acc = (
                        scores[:, ci, 0:1]
                        if nsplit == 1
                        else partial[:, ci - (nt - TAIL_CHUNKS), h : h + 1]
                    )
                    nc.vector.scalar_tensor_tensor(
                        out=et[:],
                        in0=et[:],
                        scalar=1.0,
                        in1=w_t[:, h0 : h0 + hs],
                        op0=mybir.AluOpType.mult,
                        op1=mybir.AluOpType.mult,
                        accum_out=acc,
                    )
                if split:
                    # combine the partial sums of the split chunk
                    nc.vector.tensor_reduce(
                        out=scores[:, ci, 0:1],
                        in_=partial[:, ci - (nt - TAIL_CHUNKS), 0:nsplit],
                        axis=mybir.AxisListType.X,
                        op=mybir.AluOpType.add,
                    )

            if dma_only:
                # timing variant: just ship something to the output
                prob0 = smallp.tile([BPC, S], F32, name="prob0")
                nc.vector.tensor_copy(prob0[:], et[0:BPC, 0:S])
                nc.sync.dma_start(out_d.ap(), prob0[:])
            else:
                # transpose scores -> batch-on-partitions: 4 strided PE
                # transposes, each into its OWN PSUM bank (PSUM deps are
                # bank-granular: sharing one bank would serialize every
                # exp behind the last transpose).
                # psumT[j][b, p] = scores[p, b, j] = score(b, s=128j+p)
                psumT = [
                    psump.tile([BPC, P], F32, name=f"psumT{j}", tag=f"psumT{j}")
                    for j in range(JT)
                ]
                for j in range(JT):
                    nc.tensor.transpose(
                        psumT[j][:], scores[:, j * BPC : (j + 1) * BPC, 0], id_t[:]
                    )

                # softmax over the free dim (fully local per batch row).
                # No max-subtraction: scores for this problem are bounded well
                # inside fp32 exp range (|score| < ~60), and softmax(x) is
                # mathematically identical with or without the shift.
                # exp + partial row-sums per 128-column group, so the first
                # three groups overlap the still-running dot-product stream
                # (transpose j is ready as soon as batch 7's chunk j is done).
                expt = smallp.tile([BPC, S], F32, name="expt")
                sums4 = smallp.tile([BPC, JT], F32, name="sums4")
                for j in range(JT):
                    nc.scalar.activation(
                        out=expt[:, j * P : (j + 1) * P],
                        in_=psumT[j][:],
                        func=mybir.ActivationFunctionType.Exp,
                        bias=0.0,
                        scale=1.0,
                        accum_out=sums4[:, j : j + 1],
                    )
                sums = smallp.tile([BPC, 1], F32, name="sums")
                nc.vector.tensor_reduce(
                    out=sums[:],
                    in_=sums4[:],
                    axis=mybir.AxisListType.X,
                    op=mybir.AluOpType.add,
                )
                binv = smallp.tile([BPC, 1], F32, name="binv")
                nc.vector.reciprocal(binv[:], sums[:])
                prob = smallp.tile([BPC, S], F32, name="prob")
                nc.vector.tensor_scalar_mul(prob[:], expt[:], binv[:])

                nc.sync.dma_start(out_d.ap(), prob[:])

    nc.finalize()  # Bacc: runs compile() (wait legalization, reg alloc, ...)
    return nc


def kernel(hidden, encoder_outputs, W, b, v):
    global _compiled_nc, LAST_RESULTS

    # Fold the linear layer on host (fp64 for accuracy): only the
    # encoder-input slice of W survives the softmax. Force numpy so the fold
    # never runs through a jax device backend.
    W = np.asarray(W)
    v = np.asarray(v)
    w_enc = (v.astype(np.float64) @ W[:, 2 * H :].astype(np.float64)).astype(
        np.float32
    )
    w_row = np.ascontiguousarray(w_enc[None, :])
    enc = np.ascontiguousarray(np.asarray(encoder_outputs, dtype=np.float32))

    if _compiled_nc is None:
        _compiled_nc = _build_nc()

    in_maps = [
        {
            "enc_in": enc[c * BPC : (c + 1) * BPC],
            "w_in": w_row,
        }
        for c in range(N_CORES)
    ]
    LAST_RESULTS = run_bass_kernel_spmd(
        _compiled_nc, in_maps, core_ids=list(range(N_CORES))
    )
    out = np.concatenate([r["probs_out"] for r in LAST_RESULTS.results], axis=0)
    return out.astype(np.float32)



# revision 22
# speedup vs baseline: 1.7691x; 1.7691x over previous
"""Trainium2 Bass kernel for nn_Attn_58669253263845 (sparse_attention).

Reference computation:
    hidden2 = concat(hidden[0], hidden[1])                 # [B, 2H]
    attn_input = concat(bcast(hidden2), encoder_outputs)   # [B, S, 3H]
    energy = attn_input @ W.T + b                          # [B, S, H]
    scores = energy @ v                                    # [B, S]
    out = softmax(scores, axis=S)

Everything before the softmax is linear, so
    scores[b,s] = attn_input[b,s,:] . (v @ W) + v.b
                = hidden2[b,:] . w_hid + enc[b,s,:] . w_enc + v.b
The hidden/bias terms are constant per batch row and cancel in the softmax
over S.  Hence:
    out = softmax_s(enc[b,s,:] . w_enc),  w_enc = v @ W[:, 2H:3H]

The weight fold (1024x1024 matvec) is done on host in fp64; the heavy part
(64*512 dot products of length 1024 + softmax) runs on 8 NeuronCores,
data-parallel over batch (8 batches per core).

Kernel strategy (v3):
  * enc ships as fp16 (half the HBM traffic of fp32; the kernel is
    DMA-bound and the 2^-11 input rounding moves scores by ~3e-3 -- two
    orders inside the 2e-2 gate).  Host pre-transposes each batch to
    [H, S] so the contraction dim lands on SBUF partitions.
  * ONE 1MiB DMA per batch: descriptor generation (HWDGE) is a serial
    ~650ns/DMA resource, so few big DMAs keep the stream transfer-bound
    (2913ns/batch at 360B/ns).  The batch-0 DMA is issued first so no
    other descriptor-gen delays the stream start.
  * The dots run on the PE array: for each (batch, s-chunk, h-chunk) the
    128x128 enc chunk is the *stationary* operand and the matching 128-row
    slice of w_enc is a single moving column, accumulating into a
    [128, 1] PSUM column over the 8 h-chunks.  Output-free-size-1 matmuls
    leave the PE essentially idle (and immune to p-state), so the DMA
    stream is the only real cost.
  * Softmax tail: one PSUM->SBUF copy of the [128, 32] scores, 4 PE
    transposes into a single batch-major [8, 512] PSUM tile, ONE exp on
    ACT with free-dim sum accumulation, reciprocal + scale on DVE.
  * The output leaves via a SWDGE scatter-add whose descriptors are
    PREPARED during the stream (Pool engine is idle); the tail only pays
    the trigger_dma (no 625ns HWDGE gen + no 650ns DGE latency).  The
    output region is zeroed by a small DMA early in the stream so the
    "+=" lands on zeros.
"""

import sys
import types

import numpy as np
import concourse.bacc as bacc
import concourse.bass as bass
import concourse.mybir as mybir
import concourse.tile as tile
from concourse.bass_utils import run_bass_kernel_spmd

# run_bass_kernel_spmd(trace=True) (e.g. via BASS_TRACE=1 in the env)
# imports antenv.axon_hooks, which does not exist in this container. Register
# a stub returning "no hook" so tracing degrades gracefully instead of
# raising ModuleNotFoundError.
try:
    import antenv.axon_hooks  # noqa: F401
except ImportError:
    try:
        import antenv

        _stub = types.ModuleType("antenv.axon_hooks")
        _stub.get_axon_ntff_profile_hook = lambda: None  # type: ignore[attr-defined]
        sys.modules["antenv.axon_hooks"] = _stub
        antenv.axon_hooks = _stub
    except ImportError:
        pass

N_CORES = 8
B, S, H = 64, 512, 1024
P = 128             # SBUF partitions
BPC = B // N_CORES  # batches per core = 8
JT = S // P         # s-chunks per batch = 4
HC = H // P         # h-chunks = 8

F32 = mybir.dt.float32
F16 = mybir.dt.float16
I16 = mybir.dt.int16

_compiled_nc = None
LAST_RESULTS = None  # BassKernelResults of the most recent run (for profiling)


def _build_nc(dma_only=False, compute_only=False, use_scatter=True):
    """Per-core kernel: probs[BPC, S] = softmax_s(enc[BPC, S, H] @ w_enc).

    dma_only / compute_only build crippled variants for cost attribution.
    """
    # Bacc (not raw Bass): its compile() legalizes multi-wait instructions
    # into EventSemaphore waits (TRN2 allows only 1 sync wait per inst).
    nc = bacc.Bacc("TRN2", target_bir_lowering=False, debug=False)

    # enc arrives pre-transposed per batch: [BPC, HC, P, S] fp16 where
    # enc_t[b, c, p, s] = enc[b, s, c*128+p].
    enc_d = nc.dram_tensor("enc_in", [BPC, HC, P, S], F16, kind="ExternalInput")
    # w_col[p, c] = w_enc[c*128 + p]
    w_d = nc.dram_tensor("w_in", [P, HC], F16, kind="ExternalInput")
    # scatter indices: row i -> output row i for i < BPC, -1 (ignored) after
    sidx_d = nc.dram_tensor("sidx_in", [P, 1], I16, kind="ExternalInput")
    out_d = nc.dram_tensor("probs_out", [BPC, S], F32, kind="ExternalOutput")

    enc = enc_d.ap()

    with tile.TileContext(nc) as tc:
        with (
            tc.tile_pool(name="const", bufs=1) as constp,
            tc.tile_pool(name="ebuf", bufs=BPC) as ebufp,
            tc.tile_pool(name="small", bufs=1) as smallp,
            tc.tile_pool(name="psum", bufs=1, space="PSUM") as psump,
        ):
            # Batch-0 enc DMA first: nothing delays the start of the
            # transfer stream (every other DMA's descriptor-gen then hides
            # behind a running transfer).
            ets = []
            for b in range(BPC):
                ets.append(ebufp.tile([P, HC, S], F16, name="et", tag="et"))

            def enc_dma(b):
                if compute_only:
                    nc.sync.dma_start(ets[b][0:1, 0:1, 0:1], enc[b, 0, 0:1, 0:1])
                else:
                    nc.sync.dma_start(
                        ets[b][:], enc[b].rearrange("c p s -> p c s")
                    )

            enc_dma(0)

            # w next: tiny (2KiB), gates the first matmul.
            w_col = constp.tile([P, HC], F16, name="w_col")
            nc.sync.dma_start(w_col[:], w_d.ap())

            enc_dma(1)

            # Output plumbing, all off the critical path:
            #  - zeros DMA'd over the output region (the scatter ADDs),
            #  - scatter indices,
            #  - descriptor PREP for the output scatter (SWDGE, Pool).
            prob = smallp.tile([P, S], F32, name="prob")
            nc.gpsimd.memset(prob[:], 0.0)
            if use_scatter:
                ztile = smallp.tile([BPC, S], F32, name="ztile")
                nc.gpsimd.memset(ztile[:], 0.0)
                nc.sync.dma_start(out_d.ap(), ztile[:])
                sidx = constp.tile([P, 1], I16, name="sidx")
                nc.sync.dma_start(sidx[:], sidx_d.ap())

                # prob is a full [128, S] tile (scatter shape contract);
                # rows >= BPC carry zeros (memset above) re-added to rows
                # 0-7 by tokens 8-15.
                # Completion sem must be the Tile-managed DMASW0 lane sem:
                # the end-of-kernel drain waits on it, and with prepare_only
                # the descriptor (fired by trigger_dma) increments it.
                nc.gpsimd.dma_scatter_add(
                    out_d.ap(),
                    prob[:].unsqueeze(1),  # [128,1,S]: 128*1 == roundup(16,128)
                    sidx[:],
                    16,
                    16,
                    S,
                    prepare_only=True,
                    sem=tc.sems.swdge_block()[0],
                )

            for b2 in range(2, BPC):
                enc_dma(b2)

            # identity for the PE transposes, built on-device (gpsimd is
            # otherwise idle): ones everywhere, keep only where p - f == 0.
            ones_id = constp.tile([P, P], F32, name="ones_id")
            nc.gpsimd.memset(ones_id[:], 1.0)
            id_t = constp.tile([P, P], F32, name="id_t")
            nc.gpsimd.affine_select(
                out=id_t[:],
                in_=ones_id[:],
                pattern=[[-1, P]],
                compare_op=mybir.AluOpType.is_equal,
                fill=0.0,
                channel_multiplier=1,
            )

            if dma_only:
                nc.vector.tensor_copy(prob[0:BPC, :], ets[0][0:BPC, 0, 0:S])
                if use_scatter:
                    nc.gpsimd.trigger_dma(count=None)
                else:
                    nc.sync.dma_start(out_d.ap(), prob[0:BPC, :])
            else:
                # scores ps[p, j*BPC + b] = enc[b, j*128+p, :] . w_enc,
                # accumulated over the 8 h-chunks on the PE array.
                ps = psump.tile([P, JT * BPC], F32, name="ps")
                for b in range(BPC):
                    for j in range(JT):
                        ci = j * BPC + b
                        for c in range(HC):
                            nc.tensor.matmul(
                                ps[:, ci : ci + 1],
                                ets[b][:, c, j * P : (j + 1) * P],
                                w_col[:, c : c + 1],
                                start=(c == 0),
                                stop=(c == HC - 1),
                            )

                scores = smallp.tile([P, JT * BPC], F32, name="scores")
                nc.vector.tensor_copy(scores[:], ps[:])

                # transpose scores -> batch-major [8, 512] in ONE PSUM tile
                # (one bank): a single exp covers all of it afterwards.
                psumT = psump.tile([BPC, S], F32, name="psumT")
                for j in range(JT):
                    nc.tensor.transpose(
                        psumT[:, j * P : (j + 1) * P],
                        scores[:, j * BPC : (j + 1) * BPC],
                        id_t[:],
                    )

                # softmax over the free dim (fully local per batch row).
                # No max-subtraction: scores are bounded well inside fp32
                # exp range (|score| < ~60) and softmax is shift-invariant.
                # ONE exp with free-dim sum accumulation -> sums in a
                # single ACT op.
                expt = smallp.tile([BPC, S], F32, name="expt")
                sums = smallp.tile([BPC, 1], F32, name="sums")
                nc.scalar.activation(
                    out=expt[:],
                    in_=psumT[:],
                    func=mybir.ActivationFunctionType.Exp,
                    bias=0.0,
                    scale=1.0,
                    accum_out=sums[:],
                )
                binv = smallp.tile([BPC, 1], F32, name="binv")
                nc.vector.reciprocal(binv[:], sums[:])
                nc.vector.tensor_scalar_mul(prob[0:BPC, :], expt[:], binv[:])

                if use_scatter:
                    # fire the pre-generated output descriptors (SWDGE): the
                    # trigger carries the data dep on prob, the prep did not.
                    nc.gpsimd.trigger_dma(count=None)
                    # consume the scatter's completion sem before the
                    # end-of-scope sem-range clear (race detector).
                    nc.gpsimd.wait_ge(tc.sems.swdge_block()[0], 16)
                else:
                    nc.sync.dma_start(out_d.ap(), prob[0:BPC, :])

    nc.finalize()  # Bacc: runs compile() (wait legalization, reg alloc, ...)
    return nc


def kernel(hidden, encoder_outputs, W, b, v):
    global _compiled_nc, LAST_RESULTS

    # Fold the linear layer on host (fp64 for accuracy): only the
    # encoder-input slice of W survives the softmax. Force numpy so the fold
    # never runs through a jax device backend.
    W = np.asarray(W)
    v = np.asarray(v)
    w_enc = (v.astype(np.float64) @ W[:, 2 * H :].astype(np.float64)).astype(
        np.float32
    )
    # w_col[p, c] = w_enc[c*128 + p]
    w_col = np.ascontiguousarray(w_enc.reshape(HC, P).T).astype(np.float16)
    # enc_t[b, c, p, s] = enc[b, s, c*128+p], fp16
    enc = np.asarray(encoder_outputs).astype(np.float16)
    enc_t = np.ascontiguousarray(
        enc.reshape(B, S, HC, P).transpose(0, 2, 3, 1)
    )
    # 16 scatter tokens: tokens 0-7 carry the probs; tokens 8-15 re-target
    # rows 0-7 but read prob rows 8-15, which are memset to zero on device,
    # so they add 0.  (All-valid indices keep the DMA completion semaphore
    # at its expected count of 16.)
    sidx = np.full((128, 1), -1, dtype=np.int16)
    sidx[:BPC, 0] = np.arange(BPC, dtype=np.int16)
    sidx[BPC:16, 0] = np.arange(BPC, dtype=np.int16)

    import os
    if _compiled_nc is None:
        _compiled_nc = _build_nc(
            use_scatter=os.environ.get("KERNEL_NO_SCATTER", "") != "1"
        )

    in_maps = [
        {
            "enc_in": enc_t[c * BPC : (c + 1) * BPC],
            "w_in": w_col,
            "sidx_in": sidx,
        }
        for c in range(N_CORES)
    ]
    LAST_RESULTS = run_bass_kernel_spmd(
        _compiled_nc, in_maps, core_ids=list(range(N_CORES))
    )
    out = np.concatenate([r["probs_out"] for r in LAST_RESULTS.results], axis=0)
    return out.astype(np.float32)
